# revision 13
# baseline (speedup 1.0000x reference)
"""Trainium2 Bass kernel for nn_BinaryTokenClassificationModel (segment_reduce).

Math: logits[b,i,j] = dot(segmean(1+i), w_src) + dot(segmean(513+j), w_tgt) + bias.
The dot commutes with the segment mean; this version pools FIRST on the PE and
projects per 128-segment bucket afterwards.  Tokens are relabeled on the host to
a global output row g = seg-1 (src, g 0..511) or 512+(seg-513) (tgt, g 512..1023);
g//128 picks one of 8 class-buckets, g%128 the PSUM row.  Each x tile [128,1024]
f32 is pooled by a one-hot float32r matmul (1 cycle/row at >=256 moving) into the
bucket's [128,1024] PSUM sums; when a bucket's token range ends, a single DVE
tensor_tensor_reduce multiplies by the replicated classifier row and reduces over
h, and a tiny tensor_scalar applies the host-computed 1/count (+bias for tgt).
Src bucket v IS output block v (no selector shift); tgt bucket v feeds a
stationary-broadcast matmul into rowb[:, 128v:128v+128].  Tiles are processed tgt
range first, then src range descending, so output blocks flush during the x
stream and only block 0 trails the last DMA.  The classifier row is broadcast
down 128 partitions on-device (ones-column matmul), so DMA moves only x + ~150KB.

Sharding: pure data parallel, one example (B=8) per NeuronCore (8 cores).
"""
import sys

for _p in ("/opt/trn_rl_repo", "/root/.axon_site/_ro/trn_rl_repo"):
    if _p not in sys.path:
        sys.path.append(_p)

from contextlib import ExitStack

import numpy as np

import concourse.bacc as bacc
import concourse.bass as bass
import concourse.tile as tile
from concourse import mybir
from concourse.bass_utils import run_bass_kernel_spmd

F32 = mybir.dt.float32
F32R = mybir.dt.float32r
BF16 = mybir.dt.bfloat16
P = 128
H = 1024
NB = 8               # class-buckets: 4 src (g 0..511) + 4 tgt (g 512..1023)
AL = mybir.AluOpType
SIMPLE_ORDER = False
ACTF = mybir.ActivationFunctionType


def _build_nc(NT: int, pairs, starts, stops, drains, ded_bucket, bias: float) -> bass.Bass:
    """pairs: ordered [(tile, bucket)]; starts/stops: per-pair bool; drains:
    pair index -> bucket drained right after it; ded_bucket: bucket using the
    dedicated PSUM slot (or -1)."""
    nc = bacc.Bacc("TRN2", target_bir_lowering=False, debug=False, num_devices=8)
    NP = len(pairs)
    NCC = 2 * P + NP + NB
    x_d = nc.declare_dram_parameter("x", [NT * P, H], F32, isOutput=False)
    cc_d = nc.declare_dram_parameter("consts", [P, NCC], F32, isOutput=False)
    w_d = nc.declare_dram_parameter("w", [1, 2 * H], F32, isOutput=False)
    y_d = nc.declare_dram_parameter("y", [512, 512], F32, isOutput=True)

    tile_order = []
    for (i, _q) in pairs:
        if i not in tile_order:
            tile_order.append(i)

    with tile.TileContext(nc) as tc, ExitStack() as ctx:
        xpool = ctx.enter_context(tc.tile_pool(name="xp", bufs=1))
        xstage = ctx.enter_context(tc.tile_pool(name="xs", bufs=6))
        consts = ctx.enter_context(tc.tile_pool(name="consts", bufs=1))
        segp = ctx.enter_context(tc.tile_pool(name="segp", bufs=1))
        opool = ctx.enter_context(tc.tile_pool(name="op", bufs=4))
        pp_rot = ctx.enter_context(tc.tile_pool(name="prot", bufs=2, space="PSUM"))
        pp_ded = ctx.enter_context(tc.tile_pool(name="pded", bufs=1, space="PSUM"))
        pp_row = ctx.enter_context(tc.tile_pool(name="prow", bufs=1, space="PSUM"))

        # ---- x stream first: saturate the DMA queue from t=0 ----
        # DMA into a small rotating f32 staging pool, then convert to
        # resident bf16 tiles (PE pools in bf16 at 1 cycle/row; raw-DMA f32
        # cannot legally feed an fp32r matmul).  Converts split ACT/gpsimd.
        x_tiles = {}
        for n, i in enumerate(tile_order):
            xs = xstage.tile([P, H], F32, name="xs", tag="xs")
            nc.sync.dma_start(out=xs, in_=x_d[P * i:P * (i + 1), :])
            x_tiles[i] = xpool.tile([P, H], BF16, name=f"xt{i}")
            if n % 2 == 0:
                nc.scalar.activation(out=x_tiles[i], in_=xs, func=ACTF.Copy)
            else:
                nc.vector.tensor_copy(out=x_tiles[i], in_=xs)

        # ---- small consts + classifier row on the scalar queue ----
        cc = consts.tile([P, NCC], F32)
        nc.scalar.dma_start(out=cc, in_=cc_d[:])
        # broadcast classifier row down 128 partitions: e0-row stationary
        # matmul against a zeroed tile carrying w in partition 0 (K=128)
        wz = consts.tile([P, 2 * H], F32)
        nc.vector.memset(wz, 0.0)
        nc.scalar.dma_start(out=wz[0:1, :], in_=w_d[:])
        e0 = consts.tile([P, P], F32)
        nc.vector.memset(e0, 0.0)
        nc.vector.memset(e0[0:1, :], 1.0)
        wrep = consts.tile([P, 2 * H], F32)
        for half in range(2):
            wps = pp_rot.tile([P, H], F32, name=f"wps{half}", tag="pb")
            for hh in range(2):
                nc.tensor.matmul(
                    wps[:, 512 * hh:512 * (hh + 1)], lhsT=e0,
                    rhs=wz[:, half * H + 512 * hh:half * H + 512 * (hh + 1)],
                    start=True, stop=True, skip_group_check=True)
            nc.scalar.activation(out=wrep[:, half * H:(half + 1) * H], in_=wps,
                                 func=ACTF.Copy)
        ident = cc[:, 0:P]
        iota = cc[:, P:2 * P]
        slo_tb = cc[:, 2 * P:2 * P + NP]
        rec = cc[:, 2 * P + NP:2 * P + NP + NB]

        # ---- one-hot lhsT for every (tile, bucket) pair in one fused compare ----
        cl_all = segp.tile([P, NP, P], BF16)
        nc.vector.tensor_tensor(
            out=cl_all,
            in0=iota.unsqueeze(1).to_broadcast((P, NP, P)),
            in1=slo_tb.unsqueeze(2).to_broadcast((P, NP, P)),
            op=AL.is_equal)

        junk = segp.tile([P, H], F32)
        tgt_drained = [0]
        pend_src = []
        dcols = segp.tile([P, NB], F32)
        msf = [segp.tile([P, 1], F32, name=f"msf{v}") for v in range(4)]
        mtf = [segp.tile([P, 1], F32, name=f"mtf{v}") for v in range(4)]
        rowb_ps = pp_row.tile([P, 512], F32)
        psum = {}
        lg_done = 0

        def emit_block(v):
            nonlocal lg_done
            # output block v = rowb (all tgt means) + src mean column v
            lg = opool.tile([P, 512], F32)
            if lg_done % 2 == 0:
                nc.scalar.activation(out=lg, in_=rowb_ps, func=ACTF.Identity,
                                     bias=msf[v][:, 0:1], scale=1.0)
            else:
                nc.vector.tensor_scalar(out=lg, in0=rowb_ps,
                                        scalar1=msf[v][:, 0:1], scalar2=None,
                                        op0=AL.add)
            lg_done += 1
            nc.scalar.dma_start(out=y_d[P * v:P * (v + 1), :], in_=lg)

        def drain(q):
            nonlocal lg_done
            cls = 0 if q < 4 else 1  # src | tgt
            nc.vector.tensor_tensor(out=junk, in0=psum[q],
                                    in1=wrep[:, cls * H:(cls + 1) * H], op=AL.mult)
            nc.scalar.activation(out=junk, in_=junk, func=ACTF.Copy,
                                 accum_out=dcols[:, q:q + 1])
            if cls == 0:
                v = q
                nc.vector.tensor_scalar(out=msf[v], in0=dcols[:, q:q + 1],
                                        scalar1=rec[:, q:q + 1], scalar2=None,
                                        op0=AL.mult)
                if tgt_drained[0] < 4:
                    pend_src.append(v)   # rowb incomplete; defer block add only
                else:
                    emit_block(v)
            else:
                v = q - 4
                nc.vector.tensor_scalar(out=mtf[v], in0=dcols[:, q:q + 1],
                                        scalar1=rec[:, q:q + 1], scalar2=bias,
                                        op0=AL.mult, op1=AL.add)
                nc.tensor.matmul(rowb_ps[:, P * v:P * (v + 1)],
                                 lhsT=mtf[v][:, 0:1].to_broadcast((P, P)),
                                 rhs=ident, start=True, stop=True,
                                 skip_group_check=True)
                tgt_drained[0] += 1
                if tgt_drained[0] == 4:
                    for vs in pend_src:
                        emit_block(vs)
                    pend_src.clear()

        # ---- main loop: one-hot pooling matmuls, drains at bucket closes ----
        for k, (i, q) in enumerate(pairs):
            if q not in psum:
                pool = pp_ded if q == ded_bucket else pp_rot
                psum[q] = pool.tile([P, H], F32, name=f"ps{q}", tag="pb" if pool is pp_rot else "pd")
            for hh in range(2):
                nc.tensor.matmul(
                    psum[q][:, 512 * hh:512 * (hh + 1)],
                    lhsT=cl_all[:, k, :],
                    rhs=x_tiles[i][:, 512 * hh:512 * (hh + 1)],
                    start=starts[k], stop=stops[k], skip_group_check=True)
            if k in drains:
                drain(drains[k])

    nc.compile()
    return nc


def _host_prep(inputs):
    x = np.ascontiguousarray(np.asarray(inputs["outputs"], dtype=np.float32))
    wid = np.asarray(inputs["word_ids"]).astype(np.int64)
    mask = np.asarray(inputs["attention_mask"])
    cw = np.asarray(inputs["classifier_w"], dtype=np.float32)
    bias = float(np.asarray(inputs["classifier_b"]))
    B, L, Hd = x.shape
    assert (Hd, L) == (H, 4096) and B == 8
    assert int(inputs["num_src"]) == 512 and int(inputs["num_tgt"]) == 512
    assert np.all(np.asarray(mask) == 1)

    new_seg = np.ones((B, L), np.int64)
    new_seg[:, 1:] = wid[:, 1:] != wid[:, :-1]
    seg = np.cumsum(new_seg, axis=1) - 1

    # global output row g: src seg 1..512 -> 0..511; tgt seg 513..1024 -> 512..1023
    g = np.where((seg >= 1) & (seg <= 512), seg - 1,
                 np.where((seg >= 513) & (seg <= 1024), seg - 1, -1))
    cutoff = max(int(np.nonzero(seg[b] <= 1024)[0][-1]) for b in range(B))
    NT = min((cutoff + 1 + P - 1) // P, L // P)
    Ltok = NT * P

    gq = np.where(g >= 0, g // P, -1)          # bucket 0..7
    gr = np.where(g >= 0, g % P, -1)

    # per-tile union of buckets across cores
    tile_buckets = []
    for i in range(NT):
        qs = np.unique(gq[:, i * P:(i + 1) * P])
        tile_buckets.append(sorted(int(q) for q in qs if q >= 0))

    # first tile containing any tgt bucket for any core
    T0 = next(i for i in range(NT) if any(q >= 4 for q in tile_buckets[i]))
    if SIMPLE_ORDER:
        order = list(range(NT))
    else:
        order = list(range(T0, NT)) + list(range(T0 - 1, -1, -1))
    phase = {i: (0 if i >= T0 else 1) for i in range(NT)}

    cross = [q for q in range(NB)
             if len({phase[i] for i in range(NT) if q in tile_buckets[i]}) > 1]
    assert len(cross) <= 1, f"multiple cross-phase buckets {cross}"
    ded_bucket = cross[0] if cross else -1

    pairs = [(i, q) for i in order for q in tile_buckets[i]]
    first = {}
    last = {}
    for k, (i, q) in enumerate(pairs):
        first.setdefault(q, k)
        last[q] = k
    starts = [first[q] == k for k, (i, q) in enumerate(pairs)]
    stops = [last[q] == k for k, (i, q) in enumerate(pairs)]
    drains = {last[q]: q for q in last}

    # per-core metadata: slo_tb one-hot rows and reciprocal counts
    in_maps = []
    NP = len(pairs)
    for b in range(B):
        grt = gr[b, :Ltok].reshape(NT, P).T    # [128, NT] token 128i+p at [p, i]
        gqt = gq[b, :Ltok].reshape(NT, P).T
        slo_tb = np.full((P, NP), -1.0, np.float32)
        for k, (i, q) in enumerate(pairs):
            m = gqt[:, i] == q
            slo_tb[m, k] = grt[m, i]
        counts = np.bincount(g[b][g[b] >= 0], minlength=NB * P)[:NB * P]
        recv = 1.0 / np.maximum(counts, 1.0)
        recm = recv.reshape(NB, P).T.astype(np.float32)   # [128, 8]
        iden = np.eye(P, dtype=np.float32)
        iotam = np.broadcast_to(np.arange(P, dtype=np.float32), (P, P)).copy()
        cc = np.concatenate([iden, iotam, slo_tb, recm], axis=1)
        in_maps.append({
            "x": np.ascontiguousarray(x[b, :Ltok]),
            "consts": np.ascontiguousarray(cc),
            "w": np.ascontiguousarray(cw.reshape(1, 2 * H)),
        })
    return NT, pairs, starts, stops, drains, ded_bucket, bias, in_maps


def _run(inputs, trace=False, tmpdir=None):
    NT, pairs, starts, stops, drains, ded, bias, in_maps = _host_prep(inputs)
    nc = _build_nc(NT, pairs, starts, stops, drains, ded, bias)
    res = run_bass_kernel_spmd(nc, in_maps, core_ids=list(range(8)), trace=trace, tmpdir=tmpdir)
    out = np.stack([np.asarray(r["y"], dtype=np.float32) for r in res.results])
    return out, res


def kernel(**inputs) -> np.ndarray:
    out, _ = _run(inputs, trace=False)
    return out


if __name__ == "__main__":
    # CoreSim smoke test on core 0's inputs
    import jax
    jax.config.update("jax_platforms", "cpu")
    sys.path.insert(0, "/root/problem")
    import reference as ref
    from concourse.bass_interp import CoreSim

    inputs = ref.setup_inputs()
    NT, pairs, starts, stops, drains, ded, bias, in_maps = _host_prep(inputs)
    print("NT =", NT, "NP =", len(pairs), "ded =", ded)
    print("pairs:", pairs)
    nc = _build_nc(NT, pairs, starts, stops, drains, ded, bias)
    sim = CoreSim(nc)
    for name, arr in in_maps[0].items():
        sim.tensor(name)[:] = arr
    sim.simulate()
    got = np.array(sim.tensor("y"))
    expected = np.asarray(ref.reference(**inputs))[0]
    err = np.abs(got - expected).max()
    scale = np.abs(expected).max()
    print("CoreSim abs err:", err, "rel:", err / scale)
    assert err / scale < 1e-2, "CoreSim mismatch"
    print("CORESIM PASSES")


# revision 14
# speedup vs baseline: 1.0123x; 1.0123x over previous
"""Trainium2 Bass kernel for nn_BinaryTokenClassificationModel (segment_reduce).

Math: logits[b,i,j] = dot(segmean(1+i), w_src) + dot(segmean(513+j), w_tgt) + bias.
The dot commutes with the segment mean; this version pools FIRST on the PE and
projects per 128-segment bucket afterwards.  Tokens are relabeled on the host to
a global output row g = seg-1 (src, g 0..511) or 512+(seg-513) (tgt, g 512..1023);
g//128 picks one of 8 class-buckets, g%128 the PSUM row.  Each x tile [128,1024]
f32 is pooled by a one-hot float32r matmul (1 cycle/row at >=256 moving) into the
bucket's [128,1024] PSUM sums; when a bucket's token range ends, a single DVE
tensor_tensor_reduce multiplies by the replicated classifier row and reduces over
h, and a tiny tensor_scalar applies the host-computed 1/count (+bias for tgt).
Src bucket v IS output block v (no selector shift); tgt bucket v feeds a
stationary-broadcast matmul into rowb[:, 128v:128v+128].  Tiles are processed tgt
range first, then src range descending, so output blocks flush during the x
stream and only block 0 trails the last DMA.  The classifier row is broadcast
down 128 partitions on-device (ones-column matmul), so DMA moves only x + ~150KB.

Sharding: pure data parallel, one example (B=8) per NeuronCore (8 cores).
"""
import sys

for _p in ("/opt/trn_rl_repo", "/root/.axon_site/_ro/trn_rl_repo"):
    if _p not in sys.path:
        sys.path.append(_p)

from contextlib import ExitStack

import numpy as np

import concourse.bacc as bacc
import concourse.bass as bass
import concourse.tile as tile
from concourse import mybir
from concourse.bass_utils import run_bass_kernel_spmd

F32 = mybir.dt.float32
F32R = mybir.dt.float32r
BF16 = mybir.dt.bfloat16
P = 128
H = 1024
NB = 8               # class-buckets: 4 src (g 0..511) + 4 tgt (g 512..1023)
AL = mybir.AluOpType
SIMPLE_ORDER = False
ACTF = mybir.ActivationFunctionType


def _build_nc(NT: int, pairs, starts, stops, drains, ded_bucket, bias: float) -> bass.Bass:
    """pairs: ordered [(tile, bucket)]; starts/stops: per-pair bool; drains:
    pair index -> bucket drained right after it; ded_bucket: bucket using the
    dedicated PSUM slot (or -1)."""
    nc = bacc.Bacc("TRN2", target_bir_lowering=False, debug=False, num_devices=8)
    NP = len(pairs)
    NCC = 2 * P + NP + NB
    x_d = nc.declare_dram_parameter("x", [NT * P, H], F32, isOutput=False)
    cc_d = nc.declare_dram_parameter("consts", [P, NCC], F32, isOutput=False)
    w_d = nc.declare_dram_parameter("w", [1, 2 * H], F32, isOutput=False)
    y_d = nc.declare_dram_parameter("y", [512, 512], F32, isOutput=True)

    tile_order = []
    for (i, _q) in pairs:
        if i not in tile_order:
            tile_order.append(i)

    with tile.TileContext(nc) as tc, ExitStack() as ctx:
        xpool = ctx.enter_context(tc.tile_pool(name="xp", bufs=1))
        xstage = ctx.enter_context(tc.tile_pool(name="xs", bufs=6))
        consts = ctx.enter_context(tc.tile_pool(name="consts", bufs=1))
        segp = ctx.enter_context(tc.tile_pool(name="segp", bufs=1))
        opool = ctx.enter_context(tc.tile_pool(name="op", bufs=4))
        pp_rot = ctx.enter_context(tc.tile_pool(name="prot", bufs=2, space="PSUM"))
        pp_ded = ctx.enter_context(tc.tile_pool(name="pded", bufs=1, space="PSUM"))
        pp_row = ctx.enter_context(tc.tile_pool(name="prow", bufs=1, space="PSUM"))

        # ---- x stream first: saturate the DMA queue from t=0 ----
        # ---- small consts + classifier row on the scalar queue ----
        cc = consts.tile([P, NCC], F32)
        nc.scalar.dma_start(out=cc, in_=cc_d[:])
        # broadcast classifier row down 128 partitions: e0-row stationary
        # matmul against a zeroed tile carrying w in partition 0 (K=128)
        wz = consts.tile([P, 2 * H], F32)
        nc.vector.memset(wz, 0.0)
        nc.scalar.dma_start(out=wz[0:1, :], in_=w_d[:])
        e0 = consts.tile([P, P], F32)
        nc.vector.memset(e0, 0.0)
        nc.vector.memset(e0[0:1, :], 1.0)
        wrep = consts.tile([P, 2 * H], F32)
        for half in range(2):
            wps = pp_rot.tile([P, H], F32, name=f"wps{half}", tag="pb")
            for hh in range(2):
                nc.tensor.matmul(
                    wps[:, 512 * hh:512 * (hh + 1)], lhsT=e0,
                    rhs=wz[:, half * H + 512 * hh:half * H + 512 * (hh + 1)],
                    start=True, stop=True, skip_group_check=True)
            nc.scalar.activation(out=wrep[:, half * H:(half + 1) * H], in_=wps,
                                 func=ACTF.Copy)
        ident = cc[:, 0:P]
        iota = cc[:, P:2 * P]
        slo_tb = cc[:, 2 * P:2 * P + NP]
        rec = cc[:, 2 * P + NP:2 * P + NP + NB]

        # DMA into a small rotating f32 staging pool, then convert to
        # resident bf16 tiles (PE pools in bf16 at 1 cycle/row; raw-DMA f32
        # cannot legally feed an fp32r matmul).  Converts split ACT/gpsimd.
        x_tiles = {}
        for n, i in enumerate(tile_order):
            xs = xstage.tile([P, H], F32, name="xs", tag="xs")
            nc.sync.dma_start(out=xs, in_=x_d[P * i:P * (i + 1), :])
            x_tiles[i] = xpool.tile([P, H], BF16, name=f"xt{i}")
            if n % 2 == 0:
                nc.scalar.activation(out=x_tiles[i], in_=xs, func=ACTF.Copy)
            else:
                nc.vector.tensor_copy(out=x_tiles[i], in_=xs)

        # ---- one-hot lhsT for every (tile, bucket) pair in one fused compare ----
        cl_all = segp.tile([P, NP, P], BF16)
        nc.vector.tensor_tensor(
            out=cl_all,
            in0=iota.unsqueeze(1).to_broadcast((P, NP, P)),
            in1=slo_tb.unsqueeze(2).to_broadcast((P, NP, P)),
            op=AL.is_equal)

        junk = segp.tile([P, H], F32)
        tgt_drained = [0]
        pend_src = []
        dcols = segp.tile([P, NB], F32)
        msf = [segp.tile([P, 1], F32, name=f"msf{v}") for v in range(4)]
        mtf = [segp.tile([P, 1], F32, name=f"mtf{v}") for v in range(4)]
        rowb_ps = pp_row.tile([P, 512], F32)
        psum = {}
        lg_done = 0

        def emit_block(v):
            nonlocal lg_done
            # output block v = rowb (all tgt means) + src mean column v
            lg = opool.tile([P, 512], F32)
            if lg_done % 2 == 0:
                nc.scalar.activation(out=lg, in_=rowb_ps, func=ACTF.Identity,
                                     bias=msf[v][:, 0:1], scale=1.0)
            else:
                nc.vector.tensor_scalar(out=lg, in0=rowb_ps,
                                        scalar1=msf[v][:, 0:1], scalar2=None,
                                        op0=AL.add)
            lg_done += 1
            nc.scalar.dma_start(out=y_d[P * v:P * (v + 1), :], in_=lg)

        def drain(q):
            nonlocal lg_done
            cls = 0 if q < 4 else 1  # src | tgt
            nc.vector.tensor_tensor(out=junk, in0=psum[q],
                                    in1=wrep[:, cls * H:(cls + 1) * H], op=AL.mult)
            nc.scalar.activation(out=junk, in_=junk, func=ACTF.Copy,
                                 accum_out=dcols[:, q:q + 1])
            if cls == 0:
                v = q
                nc.vector.tensor_scalar(out=msf[v], in0=dcols[:, q:q + 1],
                                        scalar1=rec[:, q:q + 1], scalar2=None,
                                        op0=AL.mult)
                if tgt_drained[0] < 4:
                    pend_src.append(v)   # rowb incomplete; defer block add only
                else:
                    emit_block(v)
            else:
                v = q - 4
                nc.vector.tensor_scalar(out=mtf[v], in0=dcols[:, q:q + 1],
                                        scalar1=rec[:, q:q + 1], scalar2=bias,
                                        op0=AL.mult, op1=AL.add)
                nc.tensor.matmul(rowb_ps[:, P * v:P * (v + 1)],
                                 lhsT=mtf[v][:, 0:1].to_broadcast((P, P)),
                                 rhs=ident, start=True, stop=True,
                                 skip_group_check=True)
                tgt_drained[0] += 1
                if tgt_drained[0] == 4:
                    for vs in pend_src:
                        emit_block(vs)
                    pend_src.clear()

        # ---- main loop: one-hot pooling matmuls, drains at bucket closes ----
        for k, (i, q) in enumerate(pairs):
            if q not in psum:
                pool = pp_ded if q == ded_bucket else pp_rot
                psum[q] = pool.tile([P, H], F32, name=f"ps{q}", tag="pb" if pool is pp_rot else "pd")
            for hh in range(2):
                nc.tensor.matmul(
                    psum[q][:, 512 * hh:512 * (hh + 1)],
                    lhsT=cl_all[:, k, :],
                    rhs=x_tiles[i][:, 512 * hh:512 * (hh + 1)],
                    start=starts[k], stop=stops[k], skip_group_check=True)
            if k in drains:
                drain(drains[k])

    nc.compile()
    return nc


def _host_prep(inputs):
    x = np.ascontiguousarray(np.asarray(inputs["outputs"], dtype=np.float32))
    wid = np.asarray(inputs["word_ids"]).astype(np.int64)
    mask = np.asarray(inputs["attention_mask"])
    cw = np.asarray(inputs["classifier_w"], dtype=np.float32)
    bias = float(np.asarray(inputs["classifier_b"]))
    B, L, Hd = x.shape
    assert (Hd, L) == (H, 4096) and B == 8
    assert int(inputs["num_src"]) == 512 and int(inputs["num_tgt"]) == 512
    assert np.all(np.asarray(mask) == 1)

    new_seg = np.ones((B, L), np.int64)
    new_seg[:, 1:] = wid[:, 1:] != wid[:, :-1]
    seg = np.cumsum(new_seg, axis=1) - 1

    # global output row g: src seg 1..512 -> 0..511; tgt seg 513..1024 -> 512..1023
    g = np.where((seg >= 1) & (seg <= 512), seg - 1,
                 np.where((seg >= 513) & (seg <= 1024), seg - 1, -1))
    cutoff = max(int(np.nonzero(seg[b] <= 1024)[0][-1]) for b in range(B))
    NT = min((cutoff + 1 + P - 1) // P, L // P)
    Ltok = NT * P

    gq = np.where(g >= 0, g // P, -1)          # bucket 0..7
    gr = np.where(g >= 0, g % P, -1)

    # per-tile union of buckets across cores
    tile_buckets = []
    for i in range(NT):
        qs = np.unique(gq[:, i * P:(i + 1) * P])
        tile_buckets.append(sorted(int(q) for q in qs if q >= 0))

    # first tile containing any tgt bucket for any core
    T0 = next(i for i in range(NT) if any(q >= 4 for q in tile_buckets[i]))
    if SIMPLE_ORDER:
        order = list(range(NT))
    else:
        order = list(range(T0, NT)) + list(range(T0 - 1, -1, -1))
    phase = {i: (0 if i >= T0 else 1) for i in range(NT)}

    cross = [q for q in range(NB)
             if len({phase[i] for i in range(NT) if q in tile_buckets[i]}) > 1]
    assert len(cross) <= 1, f"multiple cross-phase buckets {cross}"
    ded_bucket = cross[0] if cross else -1

    pairs = [(i, q) for i in order for q in tile_buckets[i]]
    first = {}
    last = {}
    for k, (i, q) in enumerate(pairs):
        first.setdefault(q, k)
        last[q] = k
    starts = [first[q] == k for k, (i, q) in enumerate(pairs)]
    stops = [last[q] == k for k, (i, q) in enumerate(pairs)]
    drains = {last[q]: q for q in last}

    # per-core metadata: slo_tb one-hot rows and reciprocal counts
    in_maps = []
    NP = len(pairs)
    for b in range(B):
        grt = gr[b, :Ltok].reshape(NT, P).T    # [128, NT] token 128i+p at [p, i]
        gqt = gq[b, :Ltok].reshape(NT, P).T
        slo_tb = np.full((P, NP), -1.0, np.float32)
        for k, (i, q) in enumerate(pairs):
            m = gqt[:, i] == q
            slo_tb[m, k] = grt[m, i]
        counts = np.bincount(g[b][g[b] >= 0], minlength=NB * P)[:NB * P]
        recv = 1.0 / np.maximum(counts, 1.0)
        recm = recv.reshape(NB, P).T.astype(np.float32)   # [128, 8]
        iden = np.eye(P, dtype=np.float32)
        iotam = np.broadcast_to(np.arange(P, dtype=np.float32), (P, P)).copy()
        cc = np.concatenate([iden, iotam, slo_tb, recm], axis=1)
        in_maps.append({
            "x": np.ascontiguousarray(x[b, :Ltok]),
            "consts": np.ascontiguousarray(cc),
            "w": np.ascontiguousarray(cw.reshape(1, 2 * H)),
        })
    return NT, pairs, starts, stops, drains, ded_bucket, bias, in_maps


def _run(inputs, trace=False, tmpdir=None):
    NT, pairs, starts, stops, drains, ded, bias, in_maps = _host_prep(inputs)
    nc = _build_nc(NT, pairs, starts, stops, drains, ded, bias)
    res = run_bass_kernel_spmd(nc, in_maps, core_ids=list(range(8)), trace=trace, tmpdir=tmpdir)
    out = np.stack([np.asarray(r["y"], dtype=np.float32) for r in res.results])
    return out, res


def kernel(**inputs) -> np.ndarray:
    out, _ = _run(inputs, trace=False)
    return out


if __name__ == "__main__":
    # CoreSim smoke test on core 0's inputs
    import jax
    jax.config.update("jax_platforms", "cpu")
    sys.path.insert(0, "/root/problem")
    import reference as ref
    from concourse.bass_interp import CoreSim

    inputs = ref.setup_inputs()
    NT, pairs, starts, stops, drains, ded, bias, in_maps = _host_prep(inputs)
    print("NT =", NT, "NP =", len(pairs), "ded =", ded)
    print("pairs:", pairs)
    nc = _build_nc(NT, pairs, starts, stops, drains, ded, bias)
    sim = CoreSim(nc)
    for name, arr in in_maps[0].items():
        sim.tensor(name)[:] = arr
    sim.simulate()
    got = np.array(sim.tensor("y"))
    expected = np.asarray(ref.reference(**inputs))[0]
    err = np.abs(got - expected).max()
    scale = np.abs(expected).max()
    print("CoreSim abs err:", err, "rel:", err / scale)
    assert err / scale < 1e-2, "CoreSim mismatch"
    print("CORESIM PASSES")
